# revision 33
# baseline (speedup 1.0000x reference)
"""Trainium2 Bass kernel for nn_DetectionLoss (YOLO-style detection loss).

Strategy (pure data parallel over 8 NeuronCores, 256 images each):
  - Host reformats det [B,125,13,13] into cell-major detT rows
    [169*B, 128] (zero-padded ch) plus a compact conf slice [B,5,169];
    gt_boxes/gt_class are passed pre-permuted object-major [128, 64, .]
    (pure layout permutations, value-independent).
  - The loss reads only ~23% of det: per core, 8 hardware dma_gather
    calls (SWDGE, 1024 rows each, 512B contiguous per row) pull the 8192
    object rows straight from HBM into an object-major [128, 64*128]
    tile.  Spreading the calls over 4 SWDGE queues overlaps their Q7
    descriptor generation (~8.6us serialized -> ~3us effective).
    One dense 865KB load covers the no-object conf term.  ~5MB HBM/core.
  - DVE does the IoU / argmax / loss terms single-shot in f32 (fewest
    instructions), the pairwise last-writer-wins dedup on [64, 4096],
    and the win-masked sums.  gt-side prep, the dense conf reduction and
    the class-channel squares (ACT) overlap the gathers.
  - Output: per-core partial sums [128, 16]; host reduces across cores.
"""
import numpy as np

GRID = 13
NA = 5
NCLS = 20
CH = 25
NCH = NA * CH          # 125
CHP = 128              # padded channel dim in detT rows
CELLS = GRID * GRID    # 169
O = 32                 # objects per image
B = 2048               # global batch
NCORES = 8
BLOC = B // NCORES     # 256 images per core
NOBJ = BLOC * O        # 8192 objects per core
J2 = NOBJ // 128       # 64 object columns
NCALL = 8              # gather calls (hardware limit ~1024 idxs per call)
IPC = BLOC // NCALL    # 32 images per call
NIC = IPC * O          # 1024 idxs per call
JC = NIC // 128        # 8 j2-columns per call

ANCHORS = np.array([1.3221, 1.73145, 3.19275, 4.00944, 5.05587,
                    8.09892, 9.47112, 4.84053, 11.2364, 10.0071],
                   dtype=np.float32)

# merged f32 const layout
CF_ID = 0
CF_IB = 128
CF_I5 = CF_IB + 512
CF_I5M = CF_I5 + 5
CF_I20 = CF_I5M + 5
CF_S2 = CF_I20 + NCLS
CF_S3 = CF_S2 + 5
CF_TRI = CF_S3 + 5
CF_W = CF_TRI + O * O

_CACHE = {}


def _make_consts():
    """Host-precomputed, data-independent constant input tensors."""
    cf = np.zeros((128, CF_W), dtype=np.float32)
    cf[:, CF_ID:CF_ID + 128] = np.eye(128, dtype=np.float32)
    # imgbase[p, cq*64 + m*8 + r] = 169 * (4m + r//2)  (img_local of idx slot)
    for cq in range(NCALL):
        for m in range(8):
            for r in range(8):
                cf[:, CF_IB + cq * 64 + m * 8 + r] = \
                    float(CELLS * (4 * m + r // 2))
    cf[:, CF_I5:CF_I5 + 5] = np.arange(5, dtype=np.float32)
    cf[:, CF_I5M:CF_I5M + 5] = np.arange(5, dtype=np.float32) - 99.0
    cf[:, CF_I20:CF_I20 + NCLS] = np.arange(NCLS, dtype=np.float32)
    cf[:, CF_S2:CF_S2 + 5] = ANCHORS[0::2] / GRID
    cf[:, CF_S3:CF_S3 + 5] = ANCHORS[1::2] / GRID
    # strict upper-triangular pair mask over (o, o2): 1.0 iff o2 > o
    tri = (np.arange(O)[None, :] > np.arange(O)[:, None]).astype(np.float32)
    cf[:, CF_TRI:CF_TRI + O * O] = tri.reshape(1, O * O)
    # fp16 selector matrices for the idx shuffle (k values <= 168, exact).
    # matmul r: out_r[i, :] = k_obj[q, :] with q = (r//2)*32 + (r%2)*16 + i%16
    sel = np.zeros((128, 8 * 128), dtype=np.float16)
    for r in range(8):
        for i in range(128):
            sel[(r // 2) * 32 + (r % 2) * 16 + (i % 16), r * 128 + i] = 1.0
    return {"c_f32": cf, "c_sel": sel}


def _build():
    """Build the Bass module (emitted once, cached)."""
    import concourse.bacc as bacc
    import concourse.tile as tile
    from concourse import mybir

    f32 = mybir.dt.float32
    f16 = mybir.dt.float16
    i16 = mybir.dt.int16
    i32 = mybir.dt.int32
    ALU = mybir.AluOpType
    AX = mybir.AxisListType
    ACT = mybir.ActivationFunctionType

    nc = bacc.Bacc(None, target_bir_lowering=False, debug=False,
                   num_swdge_queues=4)

    detT = nc.dram_tensor("detT", [BLOC * CELLS, CHP], f32,
                          kind="ExternalInput")
    conf = nc.dram_tensor("conf", [BLOC, NA, CELLS], f32,
                          kind="ExternalInput")
    gtb = nc.dram_tensor("gtb", [128, J2 * 4], f32, kind="ExternalInput")
    clsf = nc.dram_tensor("clsf", [128, J2], f32, kind="ExternalInput")
    c_f32 = nc.dram_tensor("c_f32", [128, CF_W], f32, kind="ExternalInput")
    c_sel = nc.dram_tensor("c_sel", [128, 8 * 128], f16,
                           kind="ExternalInput")
    out = nc.dram_tensor("out", [128, 16], f32, kind="ExternalOutput")

    with tile.TileContext(nc) as tc, \
         nc.allow_low_precision(reason="fp16 idx shuffle; exact small ints"):
        with tc.tile_pool(name="cpool", bufs=1) as cp, \
             tc.tile_pool(name="work", bufs=1) as wk, \
             tc.tile_pool(name="psA", bufs=2, space="PSUM") as psA:

            # ---- inputs + constants, spread across the two HWDGE queues ----
            t_gtb = wk.tile([128, J2 * 4], f32)
            t_cls = wk.tile([128, J2], f32)
            t_cf32 = cp.tile([128, CF_W], f32)
            t_sel = cp.tile([128, 8 * 128], f16)
            nc.sync.dma_start(t_gtb[:], gtb[:])
            nc.scalar.dma_start(t_sel[:], c_sel[:])
            nc.scalar.dma_start(t_cls[:], clsf[:])
            nc.sync.dma_start(t_cf32[:], c_f32[:])
            t_id = t_cf32[:, CF_ID:CF_ID + 128]
            t_ib = t_cf32[:, CF_IB:CF_IB + 512]
            c_i5 = t_cf32[:, CF_I5:CF_I5 + 5]
            c_i5m = t_cf32[:, CF_I5M:CF_I5M + 5]
            c_i20 = t_cf32[:, CF_I20:CF_I20 + NCLS]
            c_s2 = t_cf32[:, CF_S2:CF_S2 + 5]
            c_s3 = t_cf32[:, CF_S3:CF_S3 + 5]
            c_tri = t_cf32[:, CF_TRI:CF_TRI + O * O]

            gv = t_gtb[:].rearrange("p (j c) -> p j c", c=4)
            x_ap = gv[:, :, 0]
            y_ap = gv[:, :, 1]
            w_ap = gv[:, :, 2]
            h_ap = gv[:, :, 3]

            # ---- cell coords (DVE f32, object-major [128, 64]) ----
            t_mx = wk.tile([128, J2], f32)
            t_my = wk.tile([128, J2], f32)
            t_gx = wk.tile([128, J2], f32)
            t_gy = wk.tile([128, J2], f32)
            t_tx = wk.tile([128, J2], f32)
            t_ty = wk.tile([128, J2], f32)
            t_k = wk.tile([128, J2], f32)
            t_k16 = wk.tile([128, J2], f16)
            t_scr0 = wk.tile([128, J2], f32)
            nc.vector.tensor_scalar_mul(t_mx[:], x_ap, float(GRID))
            nc.vector.tensor_scalar_mul(t_my[:], y_ap, float(GRID))
            # floor(v), robust to the fp->int rounding mode:
            #   i = cvt(v); fi = cvt_back(i); gx = fi - (fi > v)
            t_i32 = wk.tile([128, J2], i32)
            for t_m_, t_g_ in ((t_mx, t_gx), (t_my, t_gy)):
                nc.vector.tensor_copy(t_i32[:], t_m_[:])
                nc.vector.tensor_copy(t_g_[:], t_i32[:])
                nc.vector.tensor_tensor(t_scr0[:], t_g_[:], t_m_[:], ALU.is_gt)
                nc.vector.tensor_sub(t_g_[:], t_g_[:], t_scr0[:])
            nc.vector.scalar_tensor_tensor(
                out=t_k[:], in0=t_gy[:], scalar=float(GRID), in1=t_gx[:],
                op0=ALU.mult, op1=ALU.add)
            nc.vector.tensor_copy(t_k16[:], t_k[:])

            # ---- gather-index shuffle into dma_gather's wrapped layout ----
            # call cq covers objects n in [1024cq, 1024cq+1024); position
            # i = n%1024 consumed at idxs[p16=i%16, jcol=i//16].  idxs value
            # = 169*img_local + k[n];  source t_k[q=(jcol%8)*16+p16, j2].
            # Selector r: out_r[i, j2] = k[(r//2)*32+(r%2)*16+i%16, j2]
            t_idxf = wk.tile([128, NCALL * 64], f32)
            for r in range(8):
                t_pr = psA.tile([128, J2], f32, space="PSUM", tag="shuf")
                nc.tensor.matmul(
                    out=t_pr[:], lhsT=t_sel[:, r * 128:(r + 1) * 128],
                    rhs=t_k16[:], start=True, stop=True)
                nc.scalar.activation(
                    t_idxf[:].rearrange("p (cq m r) -> p cq m r", cq=NCALL,
                                        r=8)[:, :, :, r],
                    t_pr[:].rearrange("p (cq m) -> p cq m", cq=NCALL),
                    ACT.Copy)
            t_idx16 = wk.tile([128, NCALL * 64], i16)
            nc.vector.tensor_add(t_idxf[:], t_idxf[:], t_ib)
            nc.vector.tensor_copy(t_idx16[:], t_idxf[:])

            # ---- object-row gathers from HBM, 4 SWDGE queues ----
            t_GT = wk.tile([128, J2 * CHP], f32)
            for cq in range(NCALL):
                nc.gpsimd.dma_gather(
                    out_ap=t_GT[:, cq * JC * CHP:(cq + 1) * JC * CHP]
                    .rearrange("p (g c) -> p g c", c=CHP),
                    in_ap=detT[cq * IPC * CELLS:(cq + 1) * IPC * CELLS],
                    idxs_ap=t_idx16[:, cq * 64:(cq + 1) * 64],
                    num_idxs=NIC, num_idxs_reg=NIC, elem_size=CHP,
                    queue_num=cq % 4)

            ghv = t_GT[:].rearrange("p (j c) -> p j c", c=CHP)
            gp = ghv[:, :, 0:NCH].rearrange("p j (a r) -> p j a r", r=CH)
            q0 = gp[:, :, :, 0]
            q1 = gp[:, :, :, 1]
            q2 = gp[:, :, :, 2]
            q3 = gp[:, :, :, 3]
            q4 = gp[:, :, :, 4]
            qclsv = gp[:, :, :, 5:CH]          # [p, J2, a, 20]

            # ---- gt-side IoU precomputation (overlaps gathers) ----
            def wj(nm):
                return wk.tile([128, J2], f32, name=nm)
            t_hw2 = wj("t_hw2"); t_hh2 = wj("t_hh2")
            t_gx0 = wj("t_gx0"); t_gy0 = wj("t_gy0")
            t_gx1 = wj("t_gx1"); t_gy1 = wj("t_gy1")
            t_a1 = wj("t_a1"); t_mm = wj("t_mm")
            nc.vector.tensor_sub(t_tx[:], t_mx[:], t_gx[:])
            nc.vector.tensor_sub(t_ty[:], t_my[:], t_gy[:])
            nc.vector.tensor_scalar_mul(t_hw2[:], w_ap, 0.5)
            nc.vector.tensor_scalar_mul(t_hh2[:], h_ap, 0.5)
            nc.vector.tensor_sub(t_gx0[:], x_ap, t_hw2[:])
            nc.vector.tensor_add(t_gx1[:], x_ap, t_hw2[:])
            nc.vector.tensor_sub(t_gy0[:], y_ap, t_hh2[:])
            nc.vector.tensor_add(t_gy1[:], y_ap, t_hh2[:])
            nc.vector.tensor_sub(t_a1[:], t_gx1[:], t_gx0[:])
            nc.vector.tensor_scalar_add(t_a1[:], t_a1[:], 1.0)
            nc.vector.tensor_sub(t_mm[:], t_gy1[:], t_gy0[:])
            nc.vector.tensor_scalar_add(t_mm[:], t_mm[:], 1.0)
            nc.vector.tensor_mul(t_a1[:], t_a1[:], t_mm[:])

            # ---- class one-hot (overlaps gathers) ----
            t_oh = wk.tile([128, J2 * NCLS], f32)
            ohv = t_oh[:].rearrange("p (j c) -> p j c", c=NCLS)
            nc.vector.tensor_tensor(
                ohv,
                t_cls[:].rearrange("p (j one) -> p j one", one=1)
                .to_broadcast([128, J2, NCLS]),
                c_i20.rearrange("p (one c) -> p one c", one=1)
                .to_broadcast([128, J2, NCLS]),
                ALU.is_equal)

            # ---- dense conf load (math emitted at the end, see below) ----
            t_stage = wk.tile([128, 16], f32)
            nc.vector.memset(t_stage[:], 0.0)
            t_cfd = wk.tile([128, 2 * NA * CELLS], f32)
            nc.scalar.dma_start(
                t_cfd[:].rearrange("p (bh a e) -> p bh a e", a=NA, e=CELLS),
                conf[:].rearrange("(bh p) a e -> p bh a e", p=128))

            # ---- class-channel squares on ACT (overlaps tail gathers) ----
            t_csq = wk.tile([128, J2 * NA * NCLS], f32)
            csqv = t_csq[:].rearrange("p (j a c) -> p j a c", a=NA, c=NCLS)
            for cq in range(NCALL):
                nc.scalar.activation(
                    csqv[:, cq * JC:(cq + 1) * JC],
                    gp[:, cq * JC:(cq + 1) * JC, :, 5:CH], ACT.Square)

            # ---- single-shot object x anchor math (f32) ----
            def w5(nm):
                return wk.tile([128, J2 * NA], f32, name=nm)
            t_iou = w5("t_iou"); t_scr = w5("t_scr"); t_scr2 = w5("t_scr2")
            t_pw = w5("t_pw"); t_ph = w5("t_ph")
            t_bx0 = w5("t_bx0"); t_by0 = w5("t_by0")
            t_bx1 = w5("t_bx1"); t_by1 = w5("t_by1")
            t_ix0 = w5("t_ix0"); t_iy0 = w5("t_iy0")
            t_inter = w5("t_inter"); t_den = w5("t_den")
            t_ohA = w5("t_ohA"); t_qcl = w5("t_qcl")
            t_mm5 = wj("t_mm5"); t_aidx = wj("t_aidx")
            t_diff = wk.tile([128, J2 * NA * 4], f32)
            t_q20 = wk.tile([128, J2 * NA * NCLS], f32)
            t_sid = wj("t_sid"); t_win = wj("t_win")
            s_csse = wj("s_csse"); s_c1 = wj("s_c1")
            s_q4 = wj("s_q4"); s_cls = wj("s_cls")

            def r5(t):
                return t[:].rearrange("p (j a) -> p j a", a=NA)

            def b5(ap2d):  # [128, J2] -> broadcast [128, J2, 5]
                return ap2d.rearrange("p (j one) -> p j one", one=1) \
                           .to_broadcast([128, J2, NA])

            def c5(ap1):  # const [128, 5] -> [128, J2, 5]
                return ap1.rearrange("p (one a) -> p one a", one=1) \
                          .to_broadcast([128, J2, NA])

            nc.vector.tensor_tensor(r5(t_pw), q2, c5(c_s2), ALU.mult)
            nc.vector.tensor_tensor(r5(t_ph), q3, c5(c_s3), ALU.mult)
            # bx0 = (px+gx)/13 - pw/2 ; by0 = (py+gy)/13 - ph/2
            nc.vector.tensor_tensor(r5(t_bx0), q0, b5(t_gx[:]), ALU.add)
            nc.vector.tensor_scalar_mul(t_bx0[:], t_bx0[:], 1.0 / GRID)
            nc.vector.scalar_tensor_tensor(
                out=t_bx0[:], in0=t_pw[:], scalar=-0.5, in1=t_bx0[:],
                op0=ALU.mult, op1=ALU.add)
            nc.vector.tensor_tensor(r5(t_by0), q1, b5(t_gy[:]), ALU.add)
            nc.vector.tensor_scalar_mul(t_by0[:], t_by0[:], 1.0 / GRID)
            nc.vector.scalar_tensor_tensor(
                out=t_by0[:], in0=t_ph[:], scalar=-0.5, in1=t_by0[:],
                op0=ALU.mult, op1=ALU.add)
            nc.vector.tensor_add(t_bx1[:], t_bx0[:], t_pw[:])
            nc.vector.tensor_add(t_by1[:], t_by0[:], t_ph[:])
            nc.vector.tensor_tensor(r5(t_ix0), r5(t_bx0), b5(t_gx0[:]),
                                    ALU.max)
            nc.vector.tensor_tensor(r5(t_iy0), r5(t_by0), b5(t_gy0[:]),
                                    ALU.max)
            nc.vector.tensor_tensor(r5(t_bx1), r5(t_bx1), b5(t_gx1[:]),
                                    ALU.min)
            nc.vector.tensor_tensor(r5(t_by1), r5(t_by1), b5(t_gy1[:]),
                                    ALU.min)
            nc.vector.tensor_sub(t_bx1[:], t_bx1[:], t_ix0[:])
            nc.vector.tensor_scalar_add(t_bx1[:], t_bx1[:], 1.0)
            nc.vector.tensor_sub(t_by1[:], t_by1[:], t_iy0[:])
            nc.vector.tensor_scalar_add(t_by1[:], t_by1[:], 1.0)
            nc.vector.tensor_mul(t_inter[:], t_bx1[:], t_by1[:])
            # a2 = (pw+1)*(ph+1); denom = a1 + a2 - inter
            nc.vector.tensor_scalar_add(t_pw[:], t_pw[:], 1.0)
            nc.vector.tensor_scalar_add(t_ph[:], t_ph[:], 1.0)
            nc.vector.tensor_mul(t_den[:], t_pw[:], t_ph[:])
            nc.vector.tensor_tensor(r5(t_den), r5(t_den), b5(t_a1[:]),
                                    ALU.add)
            nc.vector.tensor_sub(t_den[:], t_den[:], t_inter[:])
            nc.vector.reciprocal(t_den[:], t_den[:])
            nc.vector.tensor_mul(t_iou[:], t_inter[:], t_den[:])

            # ---- argmax over anchors (first max wins) ----
            nc.vector.reduce_max(t_mm5[:], r5(t_iou), axis=AX.X)
            nc.vector.tensor_tensor(
                r5(t_scr), r5(t_iou), b5(t_mm5[:]), ALU.is_equal)
            nc.vector.tensor_tensor(
                r5(t_scr2), r5(t_scr), c5(c_i5m), ALU.mult)
            nc.vector.tensor_reduce(
                t_aidx[:], r5(t_scr2), axis=AX.X, op=ALU.min)
            nc.vector.tensor_scalar_add(t_aidx[:], t_aidx[:], 99.0)

            # ---- slot id s = 169*aidx + k ----
            nc.vector.scalar_tensor_tensor(
                out=t_sid[:], in0=t_aidx[:], scalar=float(CELLS),
                in1=t_k[:], op0=ALU.mult, op1=ALU.add)

            # ---- masks: onehot(aidx) ----
            nc.vector.tensor_tensor(
                r5(t_ohA), b5(t_aidx[:]), c5(c_i5), ALU.is_equal)

            # ---- coord SSE, anchor-reduced via onehot ----
            dv = t_diff[:].rearrange("p (j a c) -> p j a c", a=NA, c=4)
            nc.vector.tensor_tensor(
                dv[:, :, :, 0], q0, b5(t_tx[:]), ALU.subtract)
            nc.vector.tensor_tensor(
                dv[:, :, :, 1], q1, b5(t_ty[:]), ALU.subtract)
            nc.vector.tensor_tensor(r5(t_scr), q2, c5(c_s2), ALU.mult)
            nc.vector.tensor_tensor(
                dv[:, :, :, 2], r5(t_scr), b5(w_ap), ALU.subtract)
            nc.vector.tensor_tensor(r5(t_scr), q3, c5(c_s3), ALU.mult)
            nc.vector.tensor_tensor(
                dv[:, :, :, 3], r5(t_scr), b5(h_ap), ALU.subtract)
            nc.vector.tensor_mul(t_diff[:], t_diff[:], t_diff[:])
            nc.vector.tensor_reduce(r5(t_scr2), dv, axis=AX.X, op=ALU.add)
            nc.vector.tensor_mul(t_scr2[:], t_scr2[:], t_ohA[:])
            nc.vector.tensor_reduce(s_csse[:], r5(t_scr2), axis=AX.X,
                                    op=ALU.add)

            # ---- conf terms: (1-q4)^2 and q4^2, anchor-reduced ----
            nc.vector.tensor_scalar(
                r5(t_scr), q4, -1.0, 1.0, ALU.mult, ALU.add)
            nc.vector.tensor_mul(t_scr[:], t_scr[:], t_scr[:])
            nc.vector.tensor_mul(t_scr[:], t_scr[:], t_ohA[:])
            nc.vector.tensor_reduce(s_c1[:], r5(t_scr), axis=AX.X, op=ALU.add)
            nc.vector.tensor_tensor(r5(t_scr), q4, q4, ALU.mult)
            nc.vector.tensor_mul(t_scr[:], t_scr[:], t_ohA[:])
            nc.vector.tensor_reduce(s_q4[:], r5(t_scr), axis=AX.X, op=ALU.add)

            # ---- class terms: S2 - 2*qcls at winner anchor ----
            q20v = t_q20[:].rearrange("p (j a c) -> p j a c", a=NA, c=NCLS)
            nc.vector.tensor_tensor(
                q20v, qclsv,
                t_oh[:].rearrange("p (j one c) -> p j one c", one=1, c=NCLS)
                .to_broadcast([128, J2, NA, NCLS]),
                ALU.mult)
            nc.vector.tensor_reduce(r5(t_qcl), q20v, axis=AX.X, op=ALU.add)
            nc.vector.tensor_reduce(r5(t_scr), csqv, axis=AX.X, op=ALU.add)
            # cls_t = S2 - 2*qcls  (the +1 handled via sum(win))
            nc.vector.scalar_tensor_tensor(
                out=t_scr2[:], in0=t_qcl[:], scalar=-2.0, in1=t_scr[:],
                op0=ALU.mult, op1=ALU.add)
            nc.vector.tensor_mul(t_scr2[:], t_scr2[:], t_ohA[:])
            nc.vector.tensor_reduce(s_cls[:], r5(t_scr2), axis=AX.X,
                                    op=ALU.add)

            # ---- single-shot last-writer-wins dedup ----
            t_sT = wk.tile([J2, 128], f32)
            t_eqp = wk.tile([J2, 4 * O * O], f32)
            t_deadT = wk.tile([J2, 128], f32)
            t_tp1 = psA.tile([J2, 128], f32, space="PSUM", tag="ded", bufs=1)
            nc.tensor.transpose(out=t_tp1[:], in_=t_sid[:], identity=t_id)
            nc.scalar.activation(t_sT[:], t_tp1[:], ACT.Copy)
            sTa = t_sT[:].rearrange("p (i o one) -> p i o one", i=4, one=1) \
                         .to_broadcast([J2, 4, O, O])
            sTb = t_sT[:].rearrange("p (i one o2) -> p i one o2", i=4, one=1) \
                         .to_broadcast([J2, 4, O, O])
            eqv = t_eqp[:].rearrange("p (i o o2) -> p i o o2", i=4, o2=O)
            nc.vector.tensor_tensor(eqv, sTa, sTb, ALU.is_equal)
            triv = c_tri[0:J2, :].rearrange(
                "p (one o o2) -> p one o o2", one=1, o2=O) \
                .to_broadcast([J2, 4, O, O])
            nc.vector.tensor_tensor(eqv, eqv, triv, ALU.mult)
            nc.vector.tensor_reduce(
                t_deadT[:].rearrange("p (i o) -> p i o", o=O),
                eqv, axis=AX.X, op=ALU.max)
            t_tp2 = psA.tile([128, J2], f32, space="PSUM", tag="ded2", bufs=1)
            nc.tensor.transpose(
                out=t_tp2[:], in_=t_deadT[:], identity=t_id[0:J2, 0:J2])
            nc.scalar.activation(t_win[:], t_tp2[:], ACT.Copy)
            nc.vector.tensor_scalar(
                t_win[:], t_win[:], -1.0, 1.0, ALU.mult, ALU.add)

            # ---- win-masked partial sums into staging ----
            t_red1 = wk.tile([128, 1], f32)

            def accw(col, stash):
                nc.vector.tensor_mul(stash[:], stash[:], t_win[:])
                nc.vector.reduce_sum(t_red1[:], stash[:], axis=AX.X)
                nc.vector.tensor_copy(t_stage[:, col:col + 1], t_red1[:])
            accw(0, s_csse)   # coord SSE (unweighted by 5)
            accw(1, s_c1)     # (1-q4)^2 at slots
            accw(2, s_q4)     # q4^2 at slots
            accw(3, s_cls)    # S2 - 2*qcls at slots
            nc.vector.reduce_sum(t_red1[:], t_win[:], axis=AX.X)
            nc.vector.tensor_copy(t_stage[:, 4:5], t_red1[:])

            # dense-conf square+reduce, emitted last so the scheduler does
            # not slot it ahead of the gather-index chain at the head
            nc.vector.tensor_mul(t_cfd[:], t_cfd[:], t_cfd[:])
            t_cfr = wk.tile([128, 2 * NA], f32)
            nc.vector.tensor_reduce(
                t_cfr[:].rearrange("p (bh a) -> p bh a", a=NA),
                t_cfd[:].rearrange("p (bh a e) -> p bh a e", a=NA, e=CELLS),
                axis=AX.X, op=ALU.add)
            nc.vector.tensor_add(
                t_stage[:, 5:10], t_cfr[:, 0:NA], t_cfr[:, NA:2 * NA])

            nc.sync.dma_start(out[:], t_stage[:])

    nc.compile()
    return nc


def _get_built():
    if "nc" not in _CACHE:
        _CACHE["nc"] = _build()
        _CACHE["consts"] = _make_consts()
    return _CACHE["nc"], _CACHE["consts"]


def _reduce_partials(P):
    """P: [ncores, 128, 16] fp32 partials -> the 4 scalar losses."""
    S = P.astype(np.float64).sum(axis=(0, 1))
    coord, confobj, confsub, clsq, wsum = S[0], S[1], S[2], S[3], S[4]
    dense = S[5:10].sum()
    obj_loss = 5.0 * coord + confobj
    no_obj_loss = 0.5 * (dense - confsub)
    conf_loss = clsq + wsum
    loss = obj_loss + no_obj_loss + conf_loss
    return (np.float32(loss), np.float32(obj_loss),
            np.float32(no_obj_loss), np.float32(conf_loss))


def kernel(detection_result, gt_boxes, gt_class):
    from concourse.bass_utils import run_bass_kernel_spmd

    nc, consts = _get_built()
    det = np.ascontiguousarray(
        np.asarray(detection_result, dtype=np.float32)).reshape(B, NCH, CELLS)
    # cell-major, channel-padded copy for the 512B-row object gathers
    detT = np.zeros((B, CELLS, CHP), dtype=np.float32)
    detT[:, :, :NCH] = det.transpose(0, 2, 1)
    detT = detT.reshape(B * CELLS, CHP)
    confs = np.ascontiguousarray(det[:, 4::CH, :])  # [B, 5, 169]
    gtb = np.asarray(gt_boxes, dtype=np.float32).reshape(NCORES, J2, 4, O, 4)
    clsv = np.asarray(gt_class).astype(np.float32).reshape(NCORES, J2, 4, O)
    # object-major permutation: row p = (b%4)*32 + o, col j2 = b//4
    gtb_om = np.ascontiguousarray(
        gtb.transpose(0, 2, 3, 1, 4).reshape(NCORES, 128, J2 * 4))
    cls_om = np.ascontiguousarray(
        clsv.transpose(0, 2, 3, 1).reshape(NCORES, 128, J2))

    in_maps = []
    for c in range(NCORES):
        sl = slice(c * BLOC, (c + 1) * BLOC)
        m = {"detT": detT[c * BLOC * CELLS:(c + 1) * BLOC * CELLS],
             "conf": confs[sl], "gtb": gtb_om[c], "clsf": cls_om[c]}
        m.update(consts)
        in_maps.append(m)

    res = run_bass_kernel_spmd(nc, in_maps, core_ids=list(range(NCORES)))
    _CACHE["last_res"] = res
    P = np.stack([res.results[c]["out"] for c in range(NCORES)])
    return _reduce_partials(P)
